# revision 18
# baseline (speedup 1.0000x reference)
"""Trainium2 Bass kernel for nn_Encoder_79585743995180 (sparse_attention).

v3 — ACT-bound pair-pipeline. Core c -> (batch n = c//2, head-group
g = c%2 owning 8 heads / 512 dims); host sums the 2 cores per batch.

Math per head (unchanged from v2, validated rel err ~7.7e-4):
  z[s,l] = k_h^T q_h   (q pre-scaled by D^-0.5, rope'd)
  wx = exp(z); colsum[s] = sum_l wx (ACT accum); rc = 1/colsum
  vs[s,:] = [v_h[s,:], 1] * rc  -> AV gives numer[d,l] rows 0-63, den row 64
  mem path: wxm = exp(zm); vm cols carry gate*mask, col 64 = mask
  attn_h = numer/den + numer_m/den_m
out = wo^T attn, emitted as two dg-halves (outT + outT2), summed on host.

v3 structural changes vs v2 (295us):
  - Heads processed in PAIRS (2g, 2g+1 share dim-group dg=g): QK/memQK
    matmuls of the two heads land on disjoint 64-row PE subarrays
    (tile_position inferred from base partitions 0/64) and run
    concurrently -> QK+memQK effective column-cycles halve.
  - The scalar engine runs ONLY the 96 exps + accum reads (~128us) and is
    the pacing engine; every copy/stage moved off it.
  - colsum accum collected per head into a [128,8] tile (one column per
    s-chunk), ONE batched reciprocal + one broadcast-AP tensor_tensor
    builds all 8 prescaled-v stationaries per head (replaces 64+64 tiny
    DVE ops).
  - v-projection staged to SBUF as 8 big strided casts into a 66-wide
    (4B-aligned) v16 layout instead of 64 small copies.
  - Combine per pair: np/den rows copied once per psum tile (np as f16
    cast at 2x), one [4,1024] reciprocal, reciprocal rows broadcast
    across partitions by DMA (not gpsimd), 3 f16 tensor ops.
  - Out-projection split into dg01/dg23 passes (summed on host with the
    core-pair sum) so only the dg3 matmuls trail the last combine.
  - Input DMAs: few large transfers, spread across the sync/scalar/
    gpsimd rings in first-use order; first matmul ~12us in.
"""

import numpy as np

import concourse.bacc as bacc
import concourse.mybir as mybir
import concourse.tile as tile
from concourse import bass_utils

F32 = mybir.dt.float32
F16 = mybir.dt.float16
NPF16 = np.float16
AF = mybir.ActivationFunctionType

L = 1024
S = 1024
N = 4
E = 1024
H = 16
D = 64
M = 512
NC = 8
HPC = 8              # heads per core
DCC = HPC * D        # 512 dims per core

_COMPILED = {}


def _build(dbg=False):
    nc = bacc.Bacc("TRN2", target_bir_lowering=False, debug=False)

    # ---- DRAM I/O (host-prechunked to [128, ...] partition layouts) ----
    xq = nc.dram_tensor("xq", [128, 8, L], F16, kind="ExternalInput").ap()
    xk = nc.dram_tensor("xk", [128, 8, L], F16, kind="ExternalInput").ap()
    xv = nc.dram_tensor("xv", [128, 8, L], F16, kind="ExternalInput").ap()
    wq = nc.dram_tensor("wq", [128, 8, DCC], F16, kind="ExternalInput").ap()
    wk = nc.dram_tensor("wk", [128, 8, DCC], F16, kind="ExternalInput").ap()
    wv = nc.dram_tensor("wv", [128, 8, DCC], F16, kind="ExternalInput").ap()
    wo = nc.dram_tensor("wo", [128, 4, E], F16, kind="ExternalInput").ap()
    cosq = nc.dram_tensor("cosq", [128, 4, L], F16, kind="ExternalInput").ap()
    sinq = nc.dram_tensor("sinq", [128, 4, L], F16, kind="ExternalInput").ap()
    cosk = nc.dram_tensor("cosk", [128, 4, L], F16, kind="ExternalInput").ap()
    sink = nc.dram_tensor("sink", [128, 4, L], F16, kind="ExternalInput").ap()
    kmem = nc.dram_tensor("kmem", [128, 4, M], F16, kind="ExternalInput").ap()
    vm = nc.dram_tensor("vm", [128, HPC, 4, 65], F16, kind="ExternalInput").ap()
    outT = nc.dram_tensor("outT", [128, 8, L], F16, kind="ExternalOutput").ap()
    outT2 = nc.dram_tensor("outT2", [128, 8, L], F16,
                           kind="ExternalOutput").ap()
    dbg_t = {}
    if dbg:
        for nm, shp, dt in (("dbg_qT", [128, L], F16),
                            ("dbg_kT", [128, L], F16),
                            ("dbg_v16", [128, 8, HPC, 66], F16),
                            ("dbg_cs", [128, 2, 8], F32),
                            ("dbg_vs", [128, 2, 8, 66], F16),
                            ("dbg_wx", [128, L], F16),
                            ("dbg_dent", [128, L], F32),
                            ("dbg_rft", [128, L], F16),
                            ("dbg_bcm", [128, L], F16),
                            ("dbg_bcx", [128, L], F16),
                            ("dbg_npm", [128, L], F16),
                            ("dbg_npx", [128, L], F16),
                            ("dbg_attn", [128, L], F16)):
            dbg_t[nm] = nc.dram_tensor(nm, shp, dt, kind="ExternalOutput").ap()

    with tile.TileContext(nc) as tc:
        with (
            tc.tile_pool(name="const", bufs=1) as const,
            tc.tile_pool(name="persist", bufs=1) as persist,
            tc.tile_pool(name="csp", bufs=1) as csp,
            tc.tile_pool(name="wexp", bufs=16) as wexpp,
            tc.tile_pool(name="wexpm", bufs=8) as wexpmp,
            tc.tile_pool(name="csac", bufs=2) as csacp,
            tc.tile_pool(name="rcp", bufs=2) as rcp,
            tc.tile_pool(name="vsp", bufs=4) as vsp,
            tc.tile_pool(name="npp", bufs=2) as npp,
            tc.tile_pool(name="rfr", bufs=2) as rfrp,
            tc.tile_pool(name="bcp", bufs=2) as bcp,
            tc.tile_pool(name="up", bufs=1) as up,
            tc.tile_pool(name="rscr", bufs=1) as rscr,
            tc.tile_pool(name="ostage", bufs=1) as ostage,
            tc.tile_pool(name="pq", bufs=2, space="PSUM") as pq,
            tc.tile_pool(name="pav", bufs=2, space="PSUM") as pavp,
        ):
            # ---- input DMAs, first-use order, 3 rings ----
            # sync ring: q path then dg0 rope
            w_sb = {}
            x_sb = {}
            for name, wsrc, xsrc, eng in (("q", wq, xq, nc.sync),
                                          ("k", wk, xk, nc.scalar),
                                          ("v", wv, xv, nc.gpsimd)):
                wt = const.tile([128, 8, DCC], F16, tag=f"w_{name}")
                eng.dma_start(out=wt, in_=wsrc)
                w_sb[name] = wt
                xt = const.tile([128, 8, L], F16, tag=f"x_{name}")
                eng.dma_start(out=xt[:, 0:4, :], in_=xsrc[:, 0:4, :])
                eng.dma_start(out=xt[:, 4:8, :], in_=xsrc[:, 4:8, :])
                x_sb[name] = xt
            kmem_sb = const.tile([128, 4, M], F16, tag="kmem")
            nc.scalar.dma_start(out=kmem_sb, in_=kmem)
            vm_sb = const.tile([128, HPC, 4, 65], F16, tag="vm")
            nc.gpsimd.dma_start(out=vm_sb, in_=vm)
            wo_sb = const.tile([128, 4, E], F16, tag="wo")
            nc.gpsimd.dma_start(out=wo_sb, in_=wo)

            cs_src = {"cq": cosq, "sq": sinq, "ck": cosk, "sk": sink}

            qT = [persist.tile([128, L], F16, tag=f"qT{i}", name=f"qT{i}")
                  for i in range(4)]
            kT = [persist.tile([128, L], F16, tag=f"kT{i}", name=f"kT{i}")
                  for i in range(4)]
            # v16: [128, sc, h, 66]; col 64 = 1.0 (den), col 65 pad (4B align)
            v16 = persist.tile([128, 8, HPC, 66], F16, tag="v16")
            attn = [persist.tile([128, L], F16, tag=f"at{i}", name=f"at{i}")
                    for i in range(4)]
            # den rows live at partitions 0/32/64/96 (32-aligned bases);
            # memset so the full-tile in-place reciprocal never reads
            # uninitialized rows
            dent = persist.tile([128, L], F32, tag="dent")
            nc.vector.memset(v16[:, :, :, 64:65], 1.0)
            nc.vector.memset(dent, 1.0)

            class ProjG:
                """One q-or-k projection group (dim-group dg)."""

                def __init__(self, dg, name):
                    self.dg, self.name = dg, name
                    self.qraw = rscr.tile([128, L], F16, tag="qraw")
                    self.ct = csp.tile([128, L], F16, tag="ct", name="ct")
                    self.st = csp.tile([128, L], F16, tag="st", name="st")
                    nc.sync.dma_start(
                        out=self.ct,
                        in_=cs_src["cq" if name == "q" else "ck"][:, dg, :])
                    nc.sync.dma_start(
                        out=self.st,
                        in_=cs_src["sq" if name == "q" else "sk"][:, dg, :])

                def chunk(self, lc):
                    ls = slice(lc * 512, (lc + 1) * 512)
                    ps = pq.tile([128, L], F32, tag="pq")
                    for kc in range(8):
                        nc.tensor.matmul(
                            ps[:, 0:512],
                            w_sb[self.name][:, kc,
                                            self.dg * 128:(self.dg + 1) * 128],
                            x_sb[self.name][:, kc, ls],
                            start=(kc == 0), stop=(kc == 7))
                    nc.vector.tensor_copy(self.qraw[:, ls], ps[:, 0:512])

                def finish(self):
                    dest = (qT if self.name == "q" else kT)[self.dg]
                    t1 = rscr.tile([128, L], F16, tag="t1")
                    nc.vector.tensor_mul(t1, self.qraw, self.ct)
                    # z = qraw * sin (sign-folded AND pre-swapped on host);
                    # t2 = partner-swap of z via gpsimd DMA copies
                    z = rscr.tile([128, L], F16, tag="z")
                    nc.vector.tensor_mul(z, self.qraw, self.st)
                    t2 = rscr.tile([128, L], F16, tag="qraw", name="t2")
                    for a in (0, 64):
                        nc.gpsimd.dma_start(
                            out=t2[a:a + 32, :], in_=z[a + 32:a + 64, :])
                        nc.gpsimd.dma_start(
                            out=t2[a + 32:a + 64, :], in_=z[a:a + 32, :])
                    nc.vector.tensor_add(dest, t1, t2)

            def emit_projv():
                # v projection: [s-rows, dims] layout; one strided cast per
                # (sp, half) into the 66-wide v16 layout
                for sp in range(4):
                    ps = pq.tile([128, L], F32, tag="pq")
                    for half in range(2):
                        st_i = sp * 2 + half
                        hs = slice(half * 512, (half + 1) * 512)
                        for kc in range(8):
                            nc.tensor.matmul(
                                ps[:, hs],
                                x_sb["v"][:, kc, st_i * 128:(st_i + 1) * 128],
                                w_sb["v"][:, kc, :],
                                start=(kc == 0), stop=(kc == 7))
                        src = ps.rearrange("p (t h d) -> p t h d",
                                           t=2, h=8)[:, half]
                        nc.vector.tensor_copy(v16[:, st_i, :, 0:64], src)

            class Pair:
                def __init__(self, g):
                    self.g = g              # dg == g; heads 2g, 2g+1
                    self.wx = {}            # (hh, sc) -> wx tile
                    self.wxm = {}           # (hh, mc) -> wxm tile
                    self.cs = {}            # hh -> [128, 8] colsum tile
                    self.vs = {}            # hh -> [128, 8, 66] prescaled v
                    self.pmain = {}         # hh -> [65, L] psum
                    self.pmem = {}
                    self.npm = None         # [128, L] f16: both heads numer
                    self.npx = None         # mem numer pair

            def emit_qk_pair(st, sc):
                # 4 MMs alternating row groups 0/64 (concurrent on the PE),
                # then the two exps with colsum accum
                dg = st.g
                pws = {}
                for hh in range(2):
                    pws[hh] = pq.tile([128, L], F32, tag="pq", name="pw")
                for lc in range(2):
                    for hh in range(2):
                        ho = hh * 64
                        nc.tensor.matmul(
                            pws[hh][:, lc * 512:(lc + 1) * 512],
                            kT[dg][ho:ho + 64, sc * 128:(sc + 1) * 128],
                            qT[dg][ho:ho + 64, lc * 512:(lc + 1) * 512],
                            start=True, stop=True)
                for hh in range(2):
                    if sc == 0:
                        st.cs[hh] = csacp.tile([128, 8], F32, tag="cs", name="cs")
                    wx = wexpp.tile([128, L], F16, tag="wx")
                    nc.scalar.activation(wx, pws[hh], AF.Exp,
                                         accum_out=st.cs[hh][:, sc:sc + 1])
                    st.wx[(hh, sc)] = wx

            def emit_rc_pair(st):
                # rc = 1/colsum (one batched recip per head), then all 8
                # prescaled-v stationaries in one broadcast tensor_tensor
                for hh in range(2):
                    h = st.g * 2 + hh
                    rc32 = rcp.tile([128, 8], F32, tag="rc32")
                    nc.vector.reciprocal_approx_fast(out=rc32, in_=st.cs[hh])
                    rc16 = rcp.tile([128, 8], F16, tag="rc16")
                    nc.vector.tensor_copy(rc16, rc32)
                    vs = vsp.tile([128, 8, 66], F16, tag="vs")
                    nc.vector.tensor_mul(
                        vs[:, :, 0:65], v16[:, :, h, 0:65],
                        rc16.unsqueeze(2).to_broadcast([128, 8, 65]))
                    st.vs[hh] = vs

            def emit_av_pair(st, sc):
                for hh in range(2):
                    if sc == 0:
                        st.pmain[hh] = pavp.tile([65, L], F32, tag="pav", name="pmain")
                    for lc in range(2):
                        nc.tensor.matmul(
                            st.pmain[hh][:, lc * 512:(lc + 1) * 512],
                            st.vs[hh][:, sc, 0:65],
                            st.wx[(hh, sc)][:, lc * 512:(lc + 1) * 512],
                            start=(sc == 0), stop=(sc == 7))

            def emit_memqk_pair(st, mc):
                dg = st.g
                pws = {}
                for hh in range(2):
                    pws[hh] = pq.tile([128, L], F32, tag="pq", name="pw")
                for lc in range(2):
                    for hh in range(2):
                        ho = hh * 64
                        nc.tensor.matmul(
                            pws[hh][:, lc * 512:(lc + 1) * 512],
                            kmem_sb[ho:ho + 64, dg, mc * 128:(mc + 1) * 128],
                            qT[dg][ho:ho + 64, lc * 512:(lc + 1) * 512],
                            start=True, stop=True)
                for hh in range(2):
                    wxm = wexpmp.tile([128, L], F16, tag="wxm")
                    nc.scalar.activation(wxm, pws[hh], AF.Exp)
                    st.wxm[(hh, mc)] = wxm

            def emit_memav_pair(st, mc):
                for hh in range(2):
                    h = st.g * 2 + hh
                    if mc == 0:
                        st.pmem[hh] = pavp.tile([65, L], F32, tag="pav", name="pmem")
                    for lc in range(2):
                        nc.tensor.matmul(
                            st.pmem[hh][:, lc * 512:(lc + 1) * 512],
                            vm_sb[:, h, mc, 0:65],
                            st.wxm[(hh, mc)][:, lc * 512:(lc + 1) * 512],
                            start=(mc == 0), stop=(mc == 3))

            def emit_release_main(st):
                # copies free the pair's pmain tiles for the mem phase
                st.npm = npp.tile([128, L], F16, tag="np")
                for hh in range(2):
                    nc.vector.tensor_copy(st.npm[hh * 64:(hh + 1) * 64, :],
                                          st.pmain[hh][0:64, :])
                    nc.vector.tensor_copy(dent[hh * 32:hh * 32 + 1, :],
                                          st.pmain[hh][64:65, :])

            def emit_release_mem(st):
                st.npx = npp.tile([128, L], F16, tag="np")
                for hh in range(2):
                    nc.vector.tensor_copy(st.npx[hh * 64:(hh + 1) * 64, :],
                                          st.pmem[hh][0:64, :])
                    nc.vector.tensor_copy(dent[64 + hh * 32:64 + hh * 32 + 1, :],
                                          st.pmem[hh][64:65, :])

            def emit_combine(st):
                # attn[dg] = npm/den[0:2] + npx/den[2:4]; reciprocal rows
                # broadcast across partitions via DMA
                if dbg and st.g == 0:
                    nc.sync.dma_start(out=dbg_t["dbg_dent"], in_=dent)
                    nc.sync.dma_start(out=dbg_t["dbg_npm"], in_=st.npm)
                    nc.sync.dma_start(out=dbg_t["dbg_npx"], in_=st.npx)
                nc.vector.reciprocal_approx_fast(out=dent, in_=dent)
                bcm = bcp.tile([128, L], F16, tag="bc")
                bcx = bcp.tile([128, L], F16, tag="bc")
                # partition_broadcast needs a partition-0-based source AND
                # destination; upper halves get a partition-shift copy
                for i, bc, scr_tag in ((0, bcm, None), (1, bcm, "um"),
                                       (2, bcx, None), (3, bcx, "ux")):
                    rr = rfrp.tile([1, L], F16, tag="rr", name="rr")
                    nc.vector.tensor_copy(rr, dent[i * 32:i * 32 + 1, :])
                    if scr_tag is None:
                        nc.gpsimd.partition_broadcast(bc[0:64, :], rr)
                    else:
                        scr = up.tile([64, L], F16, tag=scr_tag, name="scr")
                        nc.gpsimd.partition_broadcast(scr, rr)
                        nc.vector.tensor_copy(bc[64:128, :], scr)
                if dbg and st.g == 0:
                    nc.sync.dma_start(out=dbg_t["dbg_bcm"], in_=bcm)
                    nc.sync.dma_start(out=dbg_t["dbg_bcx"], in_=bcx)
                um = up.tile([128, L], F16, tag="um")
                nc.vector.tensor_mul(um, st.npm, bcm)
                ux = up.tile([128, L], F16, tag="ux")
                nc.vector.tensor_mul(ux, st.npx, bcx)
                nc.vector.tensor_add(attn[st.g], um, ux)
                if dbg and st.g == 0:
                    nc.sync.dma_start(out=dbg_t["dbg_attn"], in_=attn[0])

            def emit_oproj(oc, dgs, po):
                for lc in range(2):
                    for dg in dgs:
                        nc.tensor.matmul(
                            po[:, lc * 512:(lc + 1) * 512],
                            wo_sb[:, dg, oc * 128:(oc + 1) * 128],
                            attn[dg][:, lc * 512:(lc + 1) * 512],
                            start=(dg == dgs[0]), stop=(dg == dgs[-1]))

            def emit_ostage(po, oc, dest):
                so = ostage.tile([128, L], F16, tag="so")
                nc.vector.tensor_copy(so, po)
                nc.sync.dma_start(out=dest[:, oc, :], in_=so)

            # ---- emission ----
            # dg0 q+k first so QK pair 0 / the exp stream starts ASAP;
            # projv + later projections become PE filler inside the periods.
            g0q = ProjG(0, "q")
            g0q.chunk(0)
            g0q.chunk(1)
            g0q.finish()
            g0k = ProjG(0, "k")
            g0k.chunk(0)
            g0k.chunk(1)
            g0k.finish()

            fills = [ProjG(1, "q"), ProjG(1, "k"), ProjG(2, "q"),
                     ProjG(2, "k"), ProjG(3, "q"), ProjG(3, "k")]

            pairs = [Pair(g) for g in range(4)]

            # period -1: QK/memQK pair 0 + projv + fills g1
            cur = pairs[0]
            fa, fb = fills.pop(0), fills.pop(0)
            for sc in range(8):
                emit_qk_pair(cur, sc)
                if sc == 0:
                    emit_projv()
                if sc == 1:
                    fa.chunk(0)
                if sc == 3:
                    fa.chunk(1)
                if sc == 4:
                    fa.finish()
                    fb.chunk(0)
                if sc == 6:
                    fb.chunk(1)
                if sc == 7:
                    fb.finish()
            emit_rc_pair(cur)
            if dbg:
                nc.sync.dma_start(out=dbg_t["dbg_qT"], in_=qT[0])
                nc.sync.dma_start(out=dbg_t["dbg_kT"], in_=kT[0])
                nc.sync.dma_start(out=dbg_t["dbg_v16"], in_=v16)
                nc.sync.dma_start(out=dbg_t["dbg_wx"], in_=cur.wx[(0, 0)])
                for hh in range(2):
                    nc.sync.dma_start(out=dbg_t["dbg_cs"][:, hh, :],
                                      in_=cur.cs[hh])
                    nc.sync.dma_start(out=dbg_t["dbg_vs"][:, hh, :, :],
                                      in_=cur.vs[hh])
            for mc in range(4):
                emit_memqk_pair(cur, mc)

            # periods 0..3
            for g in range(4):
                st = pairs[g]
                nxt = pairs[g + 1] if g + 1 < 4 else None
                if fills:
                    fa, fb = fills.pop(0), fills.pop(0)
                else:
                    fa = fb = None
                for sc in range(8):
                    emit_av_pair(st, sc)
                    if nxt is not None:
                        emit_qk_pair(nxt, sc)
                    if fa is not None:
                        if sc == 1:
                            fa.chunk(0)
                        if sc == 3:
                            fa.chunk(1)
                        if sc == 4:
                            fa.finish()
                            fb.chunk(0)
                        if sc == 6:
                            fb.chunk(1)
                        if sc == 7:
                            fb.finish()
                    if g == 2 and sc % 2 == 1:
                        # out-proj pass A (dg 0+1) as PE filler
                        oc = sc // 2
                        po = pq.tile([128, L], F32, tag="pq")
                        emit_oproj(oc, (0, 1), po)
                        emit_ostage(po, oc, outT)
                if nxt is not None:
                    emit_rc_pair(nxt)
                emit_release_main(st)
                for mc in range(4):
                    emit_memav_pair(st, mc)
                    if nxt is not None:
                        emit_memqk_pair(nxt, mc)
                    if g == 2:
                        oc = 4 + mc
                        po = pq.tile([128, L], F32, tag="pq")
                        emit_oproj(oc, (0, 1), po)
                        emit_ostage(po, oc, outT)
                emit_release_mem(st)
                emit_combine(st)
                if g == 3:
                    # out-proj pass B: dg2+dg3 per oc after the last combine
                    for oc in range(8):
                        po = pq.tile([128, L], F32, tag="pq")
                        emit_oproj(oc, (2, 3), po)
                        emit_ostage(po, oc, outT2)
    nc.compile()
    return nc


def _perm64():
    p = np.empty(64, np.int64)
    p[:32] = np.arange(0, 64, 2)
    p[32:] = np.arange(1, 64, 2)
    return p


def _chunk(a, nchunk):
    # [C*128, F] -> [128, C, F]
    c128, f = a.shape
    return np.ascontiguousarray(
        a.reshape(nchunk, 128, f).transpose(1, 0, 2)).astype(NPF16)


def _prep_inputs(inputs):
    """Host-side shard prep. Returns list of per-core input dicts."""
    f = np.float32
    query = np.asarray(inputs["query"], f)
    key = np.asarray(inputs["key"], f)
    value = np.asarray(inputs["value"], f)
    W = np.asarray(inputs["in_proj_weight"], f)
    wo = np.asarray(inputs["out_proj_weight"], f)
    qp = np.asarray(inputs["qp"], f)
    kvp = np.asarray(inputs["kvp"], f)
    k_mem = np.asarray(inputs["k_mem"], f)
    v_mem = np.asarray(inputs["v_mem"], f)
    gate = np.asarray(inputs["gate_attn"], f)
    mask = np.asarray(inputs["mem_mask"]).astype(f)

    g = 1.0 / (1.0 + np.exp(-gate))
    p64 = _perm64()
    sgn = np.tile(np.concatenate(
        [np.full(32, -1.0, f), np.full(32, 1.0, f)]), HPC)

    # per-batch x, shared by the two cores of each batch
    xs = {}
    for n in range(N):
        xs[n] = tuple(
            _chunk(np.ascontiguousarray(t[:, n, :].T), 8)
            for t in (query, key, value))

    def swap32(x):
        y = np.empty_like(x)
        for hb in range(HPC):
            b = hb * 64
            y[b:b + 32] = x[b + 32:b + 64]
            y[b + 32:b + 64] = x[b:b + 32]
        return y

    in_maps = []
    for c in range(NC):
        n, grp = c // 2, c % 2
        dims = np.arange(grp * DCC, (grp + 1) * DCC)
        dims_perm = np.concatenate([dims[h * 64 + p64] for h in range(HPC)])
        gv = np.concatenate(
            [np.full(64, 1.0 - g[grp * HPC + h], f) for h in range(HPC)])

        wq_c = _chunk(np.ascontiguousarray(
            (W[:E][dims_perm] * np.float32(D ** -0.5)).T), 8)
        wk_c = _chunk(np.ascontiguousarray(W[E:2 * E][dims_perm].T), 8)
        wv_c = _chunk(np.ascontiguousarray(
            (W[2 * E:][dims] * gv[:, None]).T), 8)
        wo_c = _chunk(np.ascontiguousarray(wo[:, dims].T), 4)

        cq = _chunk(np.ascontiguousarray(qp[n][:, dims_perm, 0].T), 4)
        sq = _chunk(swap32(qp[n][:, dims_perm, 1].T * sgn[:, None]), 4)
        ck = _chunk(np.ascontiguousarray(kvp[n][:, dims_perm, 0].T), 4)
        sk = _chunk(swap32(kvp[n][:, dims_perm, 1].T * sgn[:, None]), 4)

        km = _chunk(np.ascontiguousarray(k_mem[n][dims_perm, :]), 4)

        vma = np.empty((HPC, 4, 128, 65), f)
        for h in range(HPC):
            vmh = (v_mem[n][dims[h * 64:(h + 1) * 64], :].T
                   * g[grp * HPC + h] * mask[n][:, None])      # [M, 64]
            vma[h, :, :, :64] = vmh.reshape(4, 128, 64)
            vma[h, :, :, 64] = mask[n].reshape(4, 128)
        vm_dev = np.ascontiguousarray(
            vma.transpose(2, 0, 1, 3)).astype(NPF16)           # [128,H,4,65]

        xq_c, xk_c, xv_c = xs[n]
        in_maps.append({
            "xq": xq_c, "xk": xk_c, "xv": xv_c,
            "wq": wq_c, "wk": wk_c, "wv": wv_c, "wo": wo_c,
            "cosq": cq, "sinq": sq, "cosk": ck, "sink": sk,
            "kmem": km, "vm": vm_dev,
        })
    return in_maps


def kernel(dbg=False, **inputs):
    key = ("nc", dbg)
    if key not in _COMPILED:
        _COMPILED[key] = _build(dbg)
    _COMPILED["nc"] = _COMPILED[key]
    nc = _COMPILED["nc"]
    in_maps = _prep_inputs(inputs)
    res = bass_utils.run_bass_kernel_spmd(nc, in_maps, core_ids=list(range(NC)))
    out = np.zeros((L, N, E), np.float64)
    for c, r in enumerate(res.results):
        n = c // 2
        oc = (r["outT"].astype(np.float64)
              + r["outT2"].astype(np.float64))     # [128, 8, L]
        out[:, n, :] += oc.transpose(2, 1, 0).reshape(L, E)
    out = out.astype(np.float32) + np.asarray(inputs["out_proj_bias"],
                                              np.float32)
    return out


# revision 23
# speedup vs baseline: 1.0481x; 1.0481x over previous
"""Trainium2 Bass kernel for nn_Encoder_79585743995180 (sparse_attention).

v3 — ACT-bound pair-pipeline. Core c -> (batch n = c//2, head-group
g = c%2 owning 8 heads / 512 dims); host sums the 2 cores per batch.

Math per head (unchanged from v2, validated rel err ~7.7e-4):
  z[s,l] = k_h^T q_h   (q pre-scaled by D^-0.5, rope'd)
  wx = exp(z); colsum[s] = sum_l wx (ACT accum); rc = 1/colsum
  vs[s,:] = [v_h[s,:], 1] * rc  -> AV gives numer[d,l] rows 0-63, den row 64
  mem path: wxm = exp(zm); vm cols carry gate*mask, col 64 = mask
  attn_h = numer/den + numer_m/den_m
out = wo^T attn, emitted as two dg-halves (outT + outT2), summed on host.

v3 structural changes vs v2 (295us):
  - Heads processed in PAIRS (2g, 2g+1 share dim-group dg=g): QK/memQK
    matmuls of the two heads land on disjoint 64-row PE subarrays
    (tile_position inferred from base partitions 0/64) and run
    concurrently -> QK+memQK effective column-cycles halve.
  - The scalar engine runs ONLY the 96 exps + accum reads (~128us) and is
    the pacing engine; every copy/stage moved off it.
  - colsum accum collected per head into a [128,8] tile (one column per
    s-chunk), ONE batched reciprocal + one broadcast-AP tensor_tensor
    builds all 8 prescaled-v stationaries per head (replaces 64+64 tiny
    DVE ops).
  - v-projection staged to SBUF as 8 big strided casts into a 66-wide
    (4B-aligned) v16 layout instead of 64 small copies.
  - Combine per pair: np/den rows copied once per psum tile (np as f16
    cast at 2x), one [4,1024] reciprocal, reciprocal rows broadcast
    across partitions by DMA (not gpsimd), 3 f16 tensor ops.
  - Out-projection split into dg01/dg23 passes (summed on host with the
    core-pair sum) so only the dg3 matmuls trail the last combine.
  - Input DMAs: few large transfers, spread across the sync/scalar/
    gpsimd rings in first-use order; first matmul ~12us in.
"""

import numpy as np

import concourse.bacc as bacc
import concourse.mybir as mybir
import concourse.tile as tile
from concourse import bass_utils

F32 = mybir.dt.float32
F16 = mybir.dt.float16
NPF16 = np.float16
AF = mybir.ActivationFunctionType

L = 1024
S = 1024
N = 4
E = 1024
H = 16
D = 64
M = 512
NC = 8
HPC = 8              # heads per core
DCC = HPC * D        # 512 dims per core

_COMPILED = {}


def _build(dbg=False):
    nc = bacc.Bacc("TRN2", target_bir_lowering=False, debug=False)

    # ---- DRAM I/O (host-prechunked to [128, ...] partition layouts) ----
    xq = nc.dram_tensor("xq", [128, 8, L], F16, kind="ExternalInput").ap()
    xk = nc.dram_tensor("xk", [128, 8, L], F16, kind="ExternalInput").ap()
    xv = nc.dram_tensor("xv", [128, 8, L], F16, kind="ExternalInput").ap()
    wq = nc.dram_tensor("wq", [128, 8, DCC], F16, kind="ExternalInput").ap()
    wk = nc.dram_tensor("wk", [128, 8, DCC], F16, kind="ExternalInput").ap()
    wv = nc.dram_tensor("wv", [128, 8, DCC], F16, kind="ExternalInput").ap()
    wo = nc.dram_tensor("wo", [128, 4, E], F16, kind="ExternalInput").ap()
    cosq = nc.dram_tensor("cosq", [128, 4, L], F16, kind="ExternalInput").ap()
    sinq = nc.dram_tensor("sinq", [128, 4, L], F16, kind="ExternalInput").ap()
    cosk = nc.dram_tensor("cosk", [128, 4, L], F16, kind="ExternalInput").ap()
    sink = nc.dram_tensor("sink", [128, 4, L], F16, kind="ExternalInput").ap()
    kmem = nc.dram_tensor("kmem", [128, 4, M], F16, kind="ExternalInput").ap()
    vm = nc.dram_tensor("vm", [128, HPC, 4, 65], F16, kind="ExternalInput").ap()
    outT = nc.dram_tensor("outT", [128, 8, L], F16, kind="ExternalOutput").ap()
    outT2 = nc.dram_tensor("outT2", [128, 8, L], F16,
                           kind="ExternalOutput").ap()
    outT3 = nc.dram_tensor("outT3", [128, 8, L], F16,
                           kind="ExternalOutput").ap()
    dbg_t = {}
    if dbg:
        for nm, shp, dt in (("dbg_qT", [128, L], F16),
                            ("dbg_kT", [128, L], F16),
                            ("dbg_v16", [128, 8, HPC, 66], F16),
                            ("dbg_cs", [128, 2, 8], F32),
                            ("dbg_vs", [128, 2, 8, 66], F16),
                            ("dbg_wx", [128, L], F16),
                            ("dbg_dent", [128, L], F32),
                            ("dbg_rft", [128, L], F16),
                            ("dbg_bcm", [128, L], F16),
                            ("dbg_bcx", [128, L], F16),
                            ("dbg_npm", [128, L], F16),
                            ("dbg_npx", [128, L], F16),
                            ("dbg_attn", [128, L], F16)):
            dbg_t[nm] = nc.dram_tensor(nm, shp, dt, kind="ExternalOutput").ap()

    with tile.TileContext(nc) as tc:
        with (
            tc.tile_pool(name="const", bufs=1) as const,
            tc.tile_pool(name="persist", bufs=1) as persist,
            tc.tile_pool(name="csp", bufs=1) as csp,
            tc.tile_pool(name="wexp", bufs=16) as wexpp,
            tc.tile_pool(name="wexpm", bufs=8) as wexpmp,
            tc.tile_pool(name="csac", bufs=2) as csacp,
            tc.tile_pool(name="rcp", bufs=2) as rcp,
            tc.tile_pool(name="vsp", bufs=3) as vsp,
            tc.tile_pool(name="npp", bufs=2) as npp,
            tc.tile_pool(name="rfr", bufs=1) as rfrp,
            tc.tile_pool(name="bcp", bufs=2) as bcp,
            tc.tile_pool(name="up", bufs=1) as up,
            tc.tile_pool(name="rscr", bufs=1) as rscr,
            tc.tile_pool(name="ostage", bufs=1) as ostage,
            tc.tile_pool(name="pq", bufs=2, space="PSUM") as pq,
            tc.tile_pool(name="pav", bufs=2, space="PSUM") as pavp,
        ):
            # ---- input DMAs, first-use order ----
            # q path on the sync HWDGE ring, k path on the scalar ring (in
            # parallel); per-kc-granular first chunks so the first matmuls
            # start as soon as possible. v path issued after the dg0 path.
            w_sb = {}
            x_sb = {}
            for name, wsrc, xsrc, eng in (("q", wq, xq, nc.sync),
                                          ("k", wk, xk, nc.scalar)):
                wt = const.tile([128, 8, DCC], F16, tag=f"w_{name}")
                eng.dma_start(out=wt[:, 0:1, :], in_=wsrc[:, 0:1, :])
                eng.dma_start(out=wt[:, 1:8, :], in_=wsrc[:, 1:8, :])
                w_sb[name] = wt
                xt = const.tile([128, 8, L], F16, tag=f"x_{name}")
                eng.dma_start(out=xt[:, 0:1, :], in_=xsrc[:, 0:1, :])
                eng.dma_start(out=xt[:, 1:4, :], in_=xsrc[:, 1:4, :])
                eng.dma_start(out=xt[:, 4:8, :], in_=xsrc[:, 4:8, :])
                x_sb[name] = xt
            kmem_sb = const.tile([128, 4, M], F16, tag="kmem")
            nc.scalar.dma_start(out=kmem_sb, in_=kmem)
            w_sb["v"] = const.tile([128, 8, DCC], F16, tag="w_v", name="wv")
            x_sb["v"] = const.tile([128, 8, L], F16, tag="x_v", name="xv")
            vm_sb = const.tile([128, HPC, 4, 65], F16, tag="vm")
            wo_sb = const.tile([128, 4, E], F16, tag="wo")

            def emit_vpath_dmas():
                nc.sync.dma_start(out=w_sb["v"], in_=wv)
                nc.sync.dma_start(out=x_sb["v"][:, 0:4, :], in_=xv[:, 0:4, :])
                nc.sync.dma_start(out=x_sb["v"][:, 4:8, :], in_=xv[:, 4:8, :])
                nc.gpsimd.dma_start(out=vm_sb, in_=vm)
                nc.gpsimd.dma_start(out=wo_sb, in_=wo)

            cs_src = {"cq": cosq, "sq": sinq, "ck": cosk, "sk": sink}

            qT = [persist.tile([128, L], F16, tag=f"qT{i}", name=f"qT{i}")
                  for i in range(4)]
            kT = [persist.tile([128, L], F16, tag=f"kT{i}", name=f"kT{i}")
                  for i in range(4)]
            # v16: [128, sc, h, 66]; col 64 = 1.0 (den), col 65 pad (4B align)
            v16 = persist.tile([128, 8, HPC, 66], F16, tag="v16")
            attn = [persist.tile([128, L], F16, tag=f"at{i}", name=f"at{i}")
                    for i in range(4)]
            # den rows live at partitions 0/32/64/96 (32-aligned bases);
            # memset so the full-tile in-place reciprocal never reads
            # uninitialized rows
            dent = persist.tile([64, L], F32, tag="dent")
            dentx = persist.tile([64, L], F32, tag="dentx")
            nc.vector.memset(v16[:, :, :, 64:65], 1.0)
            nc.vector.memset(dent, 1.0)
            nc.vector.memset(dentx, 1.0)

            class ProjG:
                """One q-or-k projection group (dim-group dg)."""

                def __init__(self, dg, name):
                    self.dg, self.name = dg, name
                    self.qraw = None
                    self.ct = csp.tile([128, L], F16, tag="ct", name="ct")
                    self.st = csp.tile([128, L], F16, tag="st", name="st")
                    nc.sync.dma_start(
                        out=self.ct,
                        in_=cs_src["cq" if name == "q" else "ck"][:, dg, :])
                    nc.sync.dma_start(
                        out=self.st,
                        in_=cs_src["sq" if name == "q" else "sk"][:, dg, :])

                def chunk(self, lc):
                    if self.qraw is None:
                        self.qraw = rscr.tile([128, L], F16, tag="qraw",
                                              name="qraw")
                    ls = slice(lc * 512, (lc + 1) * 512)
                    ps = pq.tile([128, L], F32, tag="pq")
                    for kc in range(8):
                        nc.tensor.matmul(
                            ps[:, 0:512],
                            w_sb[self.name][:, kc,
                                            self.dg * 128:(self.dg + 1) * 128],
                            x_sb[self.name][:, kc, ls],
                            start=(kc == 0), stop=(kc == 7))
                    nc.vector.tensor_copy(self.qraw[:, ls], ps[:, 0:512])

                def finish(self):
                    dest = (qT if self.name == "q" else kT)[self.dg]
                    t1 = rscr.tile([128, L], F16, tag="t1")
                    nc.vector.tensor_mul(t1, self.qraw, self.ct)
                    # z = qraw * sin (sign-folded AND pre-swapped on host);
                    # t2 = partner-swap of z via gpsimd DMA copies
                    z = rscr.tile([128, L], F16, tag="z")
                    nc.vector.tensor_mul(z, self.qraw, self.st)
                    t2 = rscr.tile([128, L], F16, tag="t2", name="t2")
                    for a in (0, 64):
                        nc.sync.dma_start(
                            out=t2[a:a + 32, :], in_=z[a + 32:a + 64, :])
                        nc.sync.dma_start(
                            out=t2[a + 32:a + 64, :], in_=z[a:a + 32, :])
                    nc.vector.tensor_add(dest, t1, t2)

            def emit_projv_grp(st_i):
                # v projection for one 128-row s block; one strided cast
                # into the 66-wide v16 layout
                ps = pq.tile([128, L], F32, tag="pq", name="pv")
                for kc in range(8):
                    nc.tensor.matmul(
                        ps[:, 0:512],
                        x_sb["v"][:, kc, st_i * 128:(st_i + 1) * 128],
                        w_sb["v"][:, kc, :],
                        start=(kc == 0), stop=(kc == 7))
                src = ps.rearrange("p (t h d) -> p t h d", t=2, h=8)[:, 0]
                nc.vector.tensor_copy(v16[:, st_i, :, 0:64], src)

            class Pair:
                def __init__(self, g):
                    self.g = g              # dg == g; heads 2g, 2g+1
                    self.wx = {}            # (hh, sc) -> wx tile
                    self.wxm = {}           # (hh, mc) -> wxm tile
                    self.cs = {}            # hh -> [128, 8] colsum tile
                    self.vs = {}            # hh -> [128, 8, 66] prescaled v
                    self.pmain = {}         # hh -> [65, L] psum
                    self.pmem = {}
                    self.npm = None         # [128, L] f16: both heads numer
                    self.npx = None         # mem numer pair
                    self.bcm = None
                    self.bcx = None

            def emit_qk_pair(st, sc):
                # 4 MMs alternating row groups 0/64 (concurrent on the PE),
                # then the two exps with colsum accum
                dg = st.g
                pws = {}
                for hh in range(2):
                    pws[hh] = pq.tile([128, L], F32, tag="pq", name="pw")
                for lc in range(2):
                    for hh in range(2):
                        ho = hh * 64
                        nc.tensor.matmul(
                            pws[hh][:, lc * 512:(lc + 1) * 512],
                            kT[dg][ho:ho + 64, sc * 128:(sc + 1) * 128],
                            qT[dg][ho:ho + 64, lc * 512:(lc + 1) * 512],
                            start=True, stop=True)
                for hh in range(2):
                    if sc == 0:
                        st.cs[hh] = csacp.tile([128, 8], F32, tag="cs", name="cs")
                    wx = wexpp.tile([128, L], F16, tag="wx")
                    nc.scalar.activation(wx, pws[hh], AF.Exp,
                                         accum_out=st.cs[hh][:, sc:sc + 1])
                    st.wx[(hh, sc)] = wx

            def emit_rc_pair(st):
                # rc = 1/colsum (one batched recip per head), then all 8
                # prescaled-v stationaries in one broadcast tensor_tensor
                for hh in range(2):
                    h = st.g * 2 + hh
                    rc32 = rcp.tile([128, 8], F32, tag="rc32")
                    nc.vector.reciprocal_approx_fast(out=rc32, in_=st.cs[hh])
                    rc16 = rcp.tile([128, 8], F16, tag="rc16")
                    nc.vector.tensor_copy(rc16, rc32)
                    vs = vsp.tile([128, 8, 66], F16, tag="vs")
                    nc.vector.tensor_mul(
                        vs[:, :, 0:65], v16[:, :, h, 0:65],
                        rc16.unsqueeze(2).to_broadcast([128, 8, 65]))
                    st.vs[hh] = vs

            def emit_av_pair(st, sc):
                for hh in range(2):
                    if sc == 0:
                        st.pmain[hh] = pavp.tile([65, L], F32, tag="pav", name="pmain")
                    for lc in range(2):
                        nc.tensor.matmul(
                            st.pmain[hh][:, lc * 512:(lc + 1) * 512],
                            st.vs[hh][:, sc, 0:65],
                            st.wx[(hh, sc)][:, lc * 512:(lc + 1) * 512],
                            start=(sc == 0), stop=(sc == 7))

            def emit_memqk_pair(st, mc):
                dg = st.g
                pws = {}
                for hh in range(2):
                    pws[hh] = pq.tile([128, L], F32, tag="pq", name="pw")
                for lc in range(2):
                    for hh in range(2):
                        ho = hh * 64
                        nc.tensor.matmul(
                            pws[hh][:, lc * 512:(lc + 1) * 512],
                            kmem_sb[ho:ho + 64, dg, mc * 128:(mc + 1) * 128],
                            qT[dg][ho:ho + 64, lc * 512:(lc + 1) * 512],
                            start=True, stop=True)
                for hh in range(2):
                    wxm = wexpmp.tile([128, L], F16, tag="wxm")
                    nc.scalar.activation(wxm, pws[hh], AF.Exp)
                    st.wxm[(hh, mc)] = wxm

            def emit_memav_pair(st, mc):
                for hh in range(2):
                    h = st.g * 2 + hh
                    if mc == 0:
                        st.pmem[hh] = pavp.tile([65, L], F32, tag="pav", name="pmem")
                    for lc in range(2):
                        nc.tensor.matmul(
                            st.pmem[hh][:, lc * 512:(lc + 1) * 512],
                            vm_sb[:, h, mc, 0:65],
                            st.wxm[(hh, mc)][:, lc * 512:(lc + 1) * 512],
                            start=(mc == 0), stop=(mc == 3))

            def emit_release_main(st):
                # copies free the pair's pmain tiles for the mem phase;
                # the main-side reciprocal + broadcasts run here so only
                # the mem half trails the last memAV
                st.npm = npp.tile([128, L], F16, tag="np")
                for hh in range(2):
                    nc.vector.tensor_copy(st.npm[hh * 64:(hh + 1) * 64, :],
                                          st.pmain[hh][0:64, :])
                    nc.vector.tensor_copy(dent[hh * 32:hh * 32 + 1, :],
                                          st.pmain[hh][64:65, :])
                nc.vector.reciprocal_approx_fast(out=dent, in_=dent)
                st.bcm = bcp.tile([128, L], F16, tag="bc", name="bcm")
                for i, scr_tag in ((0, None), (1, "um")):
                    rr = rfrp.tile([1, L], F16, tag="rr", name="rr")
                    nc.vector.tensor_copy(rr, dent[i * 32:i * 32 + 1, :])
                    if scr_tag is None:
                        nc.gpsimd.partition_broadcast(st.bcm[0:64, :], rr)
                    else:
                        scr = up.tile([64, L], F16, tag=scr_tag, name="scr")
                        nc.gpsimd.partition_broadcast(scr, rr)
                        nc.vector.tensor_copy(st.bcm[64:128, :], scr)

            def emit_release_mem(st):
                st.npx = npp.tile([128, L], F16, tag="np")
                for hh in range(2):
                    nc.vector.tensor_copy(st.npx[hh * 64:(hh + 1) * 64, :],
                                          st.pmem[hh][0:64, :])
                    nc.vector.tensor_copy(dentx[hh * 32:hh * 32 + 1, :],
                                          st.pmem[hh][64:65, :])
                nc.vector.reciprocal_approx_fast(out=dentx, in_=dentx)
                st.bcx = bcp.tile([128, L], F16, tag="bc", name="bcx")
                for i, scr_tag in ((0, None), (1, "ux")):
                    rr = rfrp.tile([1, L], F16, tag="rr", name="rr")
                    nc.vector.tensor_copy(rr, dentx[i * 32:i * 32 + 1, :])
                    if scr_tag is None:
                        nc.gpsimd.partition_broadcast(st.bcx[0:64, :], rr)
                    else:
                        scr = up.tile([64, L], F16, tag=scr_tag, name="scr")
                        nc.gpsimd.partition_broadcast(scr, rr)
                        nc.vector.tensor_copy(st.bcx[64:128, :], scr)

            def emit_combine(st):
                # attn[dg] = npm * bcm + npx * bcx
                if dbg and st.g == 0:
                    nc.sync.dma_start(out=dbg_t["dbg_npm"], in_=st.npm)
                    nc.sync.dma_start(out=dbg_t["dbg_npx"], in_=st.npx)
                    nc.sync.dma_start(out=dbg_t["dbg_bcm"], in_=st.bcm)
                    nc.sync.dma_start(out=dbg_t["dbg_bcx"], in_=st.bcx)
                um = up.tile([128, L], F16, tag="um")
                nc.vector.tensor_mul(um, st.npm, st.bcm)
                ux = up.tile([128, L], F16, tag="ux")
                nc.vector.tensor_mul(ux, st.npx, st.bcx)
                nc.vector.tensor_add(attn[st.g], um, ux)
                if dbg and st.g == 0:
                    nc.sync.dma_start(out=dbg_t["dbg_attn"], in_=attn[0])

            def emit_oproj(oc, dgs, po):
                for lc in range(2):
                    for dg in dgs:
                        nc.tensor.matmul(
                            po[:, lc * 512:(lc + 1) * 512],
                            wo_sb[:, dg, oc * 128:(oc + 1) * 128],
                            attn[dg][:, lc * 512:(lc + 1) * 512],
                            start=(dg == dgs[0]), stop=(dg == dgs[-1]))

            def emit_ostage(po, oc, dest):
                so = ostage.tile([128, L], F16, tag="so")
                nc.vector.tensor_copy(so, po)
                nc.sync.dma_start(out=dest[:, oc, :], in_=so)

            # ---- emission ----
            # dg0 q+k interleaved (parallel DMA rings) so QK pair 0 / the
            # exp stream starts ASAP; projv + later projections become PE
            # filler inside the periods.
            g0q = ProjG(0, "q")
            g0k = ProjG(0, "k")
            g0q.chunk(0)
            g0k.chunk(0)
            g0q.chunk(1)
            g0k.chunk(1)
            emit_vpath_dmas()
            g0q.finish()
            g0k.finish()

            fills = [(1, "q"), (1, "k"), (2, "q"),
                     (2, "k"), (3, "q"), (3, "k")]

            pairs = [Pair(g) for g in range(4)]

            # period -1: QK/memQK pair 0 + projv + fills g1
            cur = pairs[0]
            fa = fb = None
            for sc in range(8):
                if sc == 0:
                    fa = ProjG(*fills.pop(0))
                if sc == 3:
                    fb = ProjG(*fills.pop(0))
                emit_qk_pair(cur, sc)
                emit_projv_grp(sc)
                if sc == 1:
                    fa.chunk(0)
                if sc == 3:
                    fa.chunk(1)
                if sc == 4:
                    fa.finish()
                    fb.chunk(0)
                if sc == 6:
                    fb.chunk(1)
                if sc == 7:
                    fb.finish()
            emit_rc_pair(cur)
            if dbg:
                nc.sync.dma_start(out=dbg_t["dbg_qT"], in_=qT[0])
                nc.sync.dma_start(out=dbg_t["dbg_kT"], in_=kT[0])
                nc.sync.dma_start(out=dbg_t["dbg_v16"], in_=v16)
                nc.sync.dma_start(out=dbg_t["dbg_wx"], in_=cur.wx[(0, 0)])
                for hh in range(2):
                    nc.sync.dma_start(out=dbg_t["dbg_cs"][:, hh, :],
                                      in_=cur.cs[hh])
                    nc.sync.dma_start(out=dbg_t["dbg_vs"][:, hh, :, :],
                                      in_=cur.vs[hh])
            for mc in range(4):
                emit_memqk_pair(cur, mc)

            # periods 0..3
            for g in range(4):
                st = pairs[g]
                nxt = pairs[g + 1] if g + 1 < 4 else None
                fa = fb = None
                for sc in range(8):
                    if fills:
                        if sc == 0:
                            fa = ProjG(*fills.pop(0))
                        if sc == 3:
                            fb = ProjG(*fills.pop(0))
                    if nxt is not None:
                        emit_qk_pair(nxt, sc)
                    emit_av_pair(st, sc)
                    if fa is not None:
                        if sc == 1:
                            fa.chunk(0)
                        if sc == 3:
                            fa.chunk(1)
                        if sc == 4:
                            fa.finish()
                            fb.chunk(0)
                        if sc == 6:
                            fb.chunk(1)
                        if sc == 7:
                            fb.finish()
                    if g == 2 and sc % 2 == 1:
                        # out-proj pass A (dg 0+1) as PE filler
                        oc = sc // 2
                        po = pq.tile([128, L], F32, tag="pq")
                        emit_oproj(oc, (0, 1), po)
                        emit_ostage(po, oc, outT)
                    if g == 3 and sc % 2 == 1:
                        # dg2-only pass streams during period 3
                        oc = sc // 2
                        po = pq.tile([128, L], F32, tag="pq")
                        emit_oproj(oc, (2,), po)
                        emit_ostage(po, oc, outT3)
                if nxt is not None:
                    emit_rc_pair(nxt)
                emit_release_main(st)
                for mc in range(4):
                    emit_memav_pair(st, mc)
                    if nxt is not None:
                        emit_memqk_pair(nxt, mc)
                    if g == 2:
                        oc = 4 + mc
                        po = pq.tile([128, L], F32, tag="pq")
                        emit_oproj(oc, (0, 1), po)
                        emit_ostage(po, oc, outT)
                    if g == 3:
                        oc = 4 + mc
                        po = pq.tile([128, L], F32, tag="pq")
                        emit_oproj(oc, (2,), po)
                        emit_ostage(po, oc, outT3)
                emit_release_mem(st)
                emit_combine(st)
                if g == 3:
                    # tail: dg3-only matmuls after the last combine
                    for oc in range(8):
                        po = pq.tile([128, L], F32, tag="pq")
                        emit_oproj(oc, (3,), po)
                        emit_ostage(po, oc, outT2)
    nc.compile()
    return nc


def _perm64():
    p = np.empty(64, np.int64)
    p[:32] = np.arange(0, 64, 2)
    p[32:] = np.arange(1, 64, 2)
    return p


def _chunk(a, nchunk):
    # [C*128, F] -> [128, C, F]
    c128, f = a.shape
    return np.ascontiguousarray(
        a.reshape(nchunk, 128, f).transpose(1, 0, 2)).astype(NPF16)


def _prep_inputs(inputs):
    """Host-side shard prep. Returns list of per-core input dicts."""
    f = np.float32
    query = np.asarray(inputs["query"], f)
    key = np.asarray(inputs["key"], f)
    value = np.asarray(inputs["value"], f)
    W = np.asarray(inputs["in_proj_weight"], f)
    wo = np.asarray(inputs["out_proj_weight"], f)
    qp = np.asarray(inputs["qp"], f)
    kvp = np.asarray(inputs["kvp"], f)
    k_mem = np.asarray(inputs["k_mem"], f)
    v_mem = np.asarray(inputs["v_mem"], f)
    gate = np.asarray(inputs["gate_attn"], f)
    mask = np.asarray(inputs["mem_mask"]).astype(f)

    g = 1.0 / (1.0 + np.exp(-gate))
    p64 = _perm64()
    sgn = np.tile(np.concatenate(
        [np.full(32, -1.0, f), np.full(32, 1.0, f)]), HPC)

    # per-batch x, shared by the two cores of each batch
    xs = {}
    for n in range(N):
        xs[n] = tuple(
            _chunk(np.ascontiguousarray(t[:, n, :].T), 8)
            for t in (query, key, value))

    def swap32(x):
        y = np.empty_like(x)
        for hb in range(HPC):
            b = hb * 64
            y[b:b + 32] = x[b + 32:b + 64]
            y[b + 32:b + 64] = x[b:b + 32]
        return y

    in_maps = []
    for c in range(NC):
        n, grp = c // 2, c % 2
        dims = np.arange(grp * DCC, (grp + 1) * DCC)
        dims_perm = np.concatenate([dims[h * 64 + p64] for h in range(HPC)])
        gv = np.concatenate(
            [np.full(64, 1.0 - g[grp * HPC + h], f) for h in range(HPC)])

        wq_c = _chunk(np.ascontiguousarray(
            (W[:E][dims_perm] * np.float32(D ** -0.5)).T), 8)
        wk_c = _chunk(np.ascontiguousarray(W[E:2 * E][dims_perm].T), 8)
        wv_c = _chunk(np.ascontiguousarray(
            (W[2 * E:][dims] * gv[:, None]).T), 8)
        wo_c = _chunk(np.ascontiguousarray(wo[:, dims].T), 4)

        cq = _chunk(np.ascontiguousarray(qp[n][:, dims_perm, 0].T), 4)
        sq = _chunk(swap32(qp[n][:, dims_perm, 1].T * sgn[:, None]), 4)
        ck = _chunk(np.ascontiguousarray(kvp[n][:, dims_perm, 0].T), 4)
        sk = _chunk(swap32(kvp[n][:, dims_perm, 1].T * sgn[:, None]), 4)

        km = _chunk(np.ascontiguousarray(k_mem[n][dims_perm, :]), 4)

        vma = np.empty((HPC, 4, 128, 65), f)
        for h in range(HPC):
            vmh = (v_mem[n][dims[h * 64:(h + 1) * 64], :].T
                   * g[grp * HPC + h] * mask[n][:, None])      # [M, 64]
            vma[h, :, :, :64] = vmh.reshape(4, 128, 64)
            vma[h, :, :, 64] = mask[n].reshape(4, 128)
        vm_dev = np.ascontiguousarray(
            vma.transpose(2, 0, 1, 3)).astype(NPF16)           # [128,H,4,65]

        xq_c, xk_c, xv_c = xs[n]
        in_maps.append({
            "xq": xq_c, "xk": xk_c, "xv": xv_c,
            "wq": wq_c, "wk": wk_c, "wv": wv_c, "wo": wo_c,
            "cosq": cq, "sinq": sq, "cosk": ck, "sink": sk,
            "kmem": km, "vm": vm_dev,
        })
    return in_maps


def kernel(dbg=False, **inputs):
    key = ("nc", dbg)
    if key not in _COMPILED:
        _COMPILED[key] = _build(dbg)
    _COMPILED["nc"] = _COMPILED[key]
    nc = _COMPILED["nc"]
    in_maps = _prep_inputs(inputs)
    res = bass_utils.run_bass_kernel_spmd(nc, in_maps, core_ids=list(range(NC)))
    out = np.zeros((L, N, E), np.float64)
    for c, r in enumerate(res.results):
        n = c // 2
        oc = (r["outT"].astype(np.float64)
              + r["outT2"].astype(np.float64)
              + r["outT3"].astype(np.float64))     # [128, 8, L]
        out[:, n, :] += oc.transpose(2, 1, 0).reshape(L, E)
    out = out.astype(np.float32) + np.asarray(inputs["out_proj_bias"],
                                              np.float32)
    return out


# revision 24
# speedup vs baseline: 1.1448x; 1.0923x over previous
"""Trainium2 Bass kernel for nn_Encoder_79585743995180 (sparse_attention).

v3 — ACT-bound pair-pipeline. Core c -> (batch n = c//2, head-group
g = c%2 owning 8 heads / 512 dims); host sums the 2 cores per batch.

Math per head (unchanged from v2, validated rel err ~7.7e-4):
  z[s,l] = k_h^T q_h   (q pre-scaled by D^-0.5, rope'd)
  wx = exp(z); colsum[s] = sum_l wx (ACT accum); rc = 1/colsum
  vs[s,:] = [v_h[s,:], 1] * rc  -> AV gives numer[d,l] rows 0-63, den row 64
  mem path: wxm = exp(zm); vm cols carry gate*mask, col 64 = mask
  attn_h = numer/den + numer_m/den_m
out = wo^T attn, emitted as two dg-halves (outT + outT2), summed on host.

v3 structural changes vs v2 (295us):
  - Heads processed in PAIRS (2g, 2g+1 share dim-group dg=g): QK/memQK
    matmuls of the two heads land on disjoint 64-row PE subarrays
    (tile_position inferred from base partitions 0/64) and run
    concurrently -> QK+memQK effective column-cycles halve.
  - The scalar engine runs ONLY the 96 exps + accum reads (~128us) and is
    the pacing engine; every copy/stage moved off it.
  - colsum accum collected per head into a [128,8] tile (one column per
    s-chunk), ONE batched reciprocal + one broadcast-AP tensor_tensor
    builds all 8 prescaled-v stationaries per head (replaces 64+64 tiny
    DVE ops).
  - v-projection staged to SBUF as 8 big strided casts into a 66-wide
    (4B-aligned) v16 layout instead of 64 small copies.
  - Combine per pair: np/den rows copied once per psum tile (np as f16
    cast at 2x), one [4,1024] reciprocal, reciprocal rows broadcast
    across partitions by DMA (not gpsimd), 3 f16 tensor ops.
  - Out-projection split into dg01/dg23 passes (summed on host with the
    core-pair sum) so only the dg3 matmuls trail the last combine.
  - Input DMAs: few large transfers, spread across the sync/scalar/
    gpsimd rings in first-use order; first matmul ~12us in.
"""

import numpy as np

import concourse.bacc as bacc
import concourse.mybir as mybir
import concourse.tile as tile
from concourse import bass_utils

F32 = mybir.dt.float32
F16 = mybir.dt.float16
NPF16 = np.float16
AF = mybir.ActivationFunctionType

L = 1024
S = 1024
N = 4
E = 1024
H = 16
D = 64
M = 512
NC = 8
HPC = 8              # heads per core
DCC = HPC * D        # 512 dims per core

_COMPILED = {}


def _build(dbg=False):
    nc = bacc.Bacc("TRN2", target_bir_lowering=False, debug=False)

    # ---- DRAM I/O (host-prechunked to [128, ...] partition layouts) ----
    xq = nc.dram_tensor("xq", [128, 8, L], F16, kind="ExternalInput").ap()
    xk = nc.dram_tensor("xk", [128, 8, L], F16, kind="ExternalInput").ap()
    xv = nc.dram_tensor("xv", [128, 8, L], F16, kind="ExternalInput").ap()
    wq = nc.dram_tensor("wq", [128, 8, DCC], F16, kind="ExternalInput").ap()
    wk = nc.dram_tensor("wk", [128, 8, DCC], F16, kind="ExternalInput").ap()
    wv = nc.dram_tensor("wv", [128, 8, DCC], F16, kind="ExternalInput").ap()
    wo = nc.dram_tensor("wo", [128, 4, E], F16, kind="ExternalInput").ap()
    cosq = nc.dram_tensor("cosq", [128, 4, L], F16, kind="ExternalInput").ap()
    sinq = nc.dram_tensor("sinq", [128, 4, L], F16, kind="ExternalInput").ap()
    cosk = nc.dram_tensor("cosk", [128, 4, L], F16, kind="ExternalInput").ap()
    sink = nc.dram_tensor("sink", [128, 4, L], F16, kind="ExternalInput").ap()
    kmem = nc.dram_tensor("kmem", [128, 4, M], F16, kind="ExternalInput").ap()
    vm = nc.dram_tensor("vm", [128, HPC, 4, 65], F16, kind="ExternalInput").ap()
    outT = nc.dram_tensor("outT", [128, 8, L], F16, kind="ExternalOutput").ap()
    outT2 = nc.dram_tensor("outT2", [128, 8, L], F16,
                           kind="ExternalOutput").ap()
    outT3 = nc.dram_tensor("outT3", [128, 8, L], F16,
                           kind="ExternalOutput").ap()
    dbg_t = {}
    if dbg:
        for nm, shp, dt in (("dbg_qT", [128, L], F16),
                            ("dbg_kT", [128, L], F16),
                            ("dbg_v16", [128, 8, HPC, 66], F16),
                            ("dbg_cs", [128, 2, 8], F32),
                            ("dbg_vs", [128, 2, 8, 66], F16),
                            ("dbg_wx", [128, L], F16),
                            ("dbg_dent", [128, L], F32),
                            ("dbg_rft", [128, L], F16),
                            ("dbg_bcm", [128, L], F16),
                            ("dbg_bcx", [128, L], F16),
                            ("dbg_npm", [128, L], F16),
                            ("dbg_npx", [128, L], F16),
                            ("dbg_attn", [128, L], F16)):
            dbg_t[nm] = nc.dram_tensor(nm, shp, dt, kind="ExternalOutput").ap()

    with tile.TileContext(nc) as tc:
        with (
            tc.tile_pool(name="const", bufs=1) as const,
            tc.tile_pool(name="persist", bufs=1) as persist,
            tc.tile_pool(name="csp", bufs=1) as csp,
            tc.tile_pool(name="wexp", bufs=16) as wexpp,
            tc.tile_pool(name="wexpm", bufs=8) as wexpmp,
            tc.tile_pool(name="csac", bufs=2) as csacp,
            tc.tile_pool(name="rcp", bufs=2) as rcp,
            tc.tile_pool(name="vsp", bufs=2) as vsp,
            tc.tile_pool(name="npp", bufs=2) as npp,
            tc.tile_pool(name="rfr", bufs=1) as rfrp,
            tc.tile_pool(name="bcp", bufs=2) as bcp,
            tc.tile_pool(name="up", bufs=1) as up,
            tc.tile_pool(name="rscr", bufs=1) as rscr,
            tc.tile_pool(name="ostage", bufs=2) as ostage,
            tc.tile_pool(name="pq", bufs=2, space="PSUM") as pq,
            tc.tile_pool(name="pav", bufs=2, space="PSUM") as pavp,
        ):
            # ---- input DMAs, first-use order ----
            # q path on the sync HWDGE ring, k path on the scalar ring (in
            # parallel); per-kc-granular first chunks so the first matmuls
            # start as soon as possible. v path issued after the dg0 path.
            w_sb = {}
            x_sb = {}
            for name, wsrc, xsrc, eng in (("q", wq, xq, nc.sync),
                                          ("k", wk, xk, nc.scalar)):
                wt = const.tile([128, 8, DCC], F16, tag=f"w_{name}")
                xt = const.tile([128, 8, L], F16, tag=f"x_{name}")
                eng.dma_start(out=wt[:, 0:1, :], in_=wsrc[:, 0:1, :])
                eng.dma_start(out=xt[:, 0:1, :], in_=xsrc[:, 0:1, :])
                eng.dma_start(out=wt[:, 1:8, :], in_=wsrc[:, 1:8, :])
                eng.dma_start(out=xt[:, 1:4, :], in_=xsrc[:, 1:4, :])
                eng.dma_start(out=xt[:, 4:8, :], in_=xsrc[:, 4:8, :])
                w_sb[name] = wt
                x_sb[name] = xt
            kmem_sb = const.tile([128, 4, M], F16, tag="kmem")
            nc.scalar.dma_start(out=kmem_sb, in_=kmem)
            w_sb["v"] = const.tile([128, 8, DCC], F16, tag="w_v", name="wv")
            x_sb["v"] = const.tile([128, 8, L], F16, tag="x_v", name="xv")
            vm_sb = const.tile([128, HPC, 4, 65], F16, tag="vm")
            wo_sb = const.tile([128, 4, E], F16, tag="wo")

            def emit_vpath_dmas():
                nc.sync.dma_start(out=w_sb["v"], in_=wv)
                nc.sync.dma_start(out=x_sb["v"][:, 0:4, :], in_=xv[:, 0:4, :])
                nc.sync.dma_start(out=x_sb["v"][:, 4:8, :], in_=xv[:, 4:8, :])
                nc.gpsimd.dma_start(out=vm_sb, in_=vm)
                nc.gpsimd.dma_start(out=wo_sb, in_=wo)

            cs_src = {"cq": cosq, "sq": sinq, "ck": cosk, "sk": sink}

            qT = [persist.tile([128, L], F16, tag=f"qT{i}", name=f"qT{i}")
                  for i in range(4)]
            kT = [persist.tile([128, L], F16, tag=f"kT{i}", name=f"kT{i}")
                  for i in range(4)]
            # v16: [128, sc, h, 66]; col 64 = 1.0 (den), col 65 pad (4B align)
            v16 = persist.tile([128, 8, HPC, 66], F16, tag="v16")
            attn = [persist.tile([128, L], F16, tag=f"at{i}", name=f"at{i}")
                    for i in range(4)]
            # den rows live at partitions 0/32/64/96 (32-aligned bases);
            # memset so the full-tile in-place reciprocal never reads
            # uninitialized rows
            dent = persist.tile([64, L], F32, tag="dent")
            dentx = persist.tile([64, L], F32, tag="dentx")
            nc.vector.memset(v16[:, :, :, 64:65], 1.0)
            nc.vector.memset(dent, 1.0)
            nc.vector.memset(dentx, 1.0)

            class ProjG:
                """One q-or-k projection group (dim-group dg)."""

                def __init__(self, dg, name):
                    self.dg, self.name = dg, name
                    self.qraw = None
                    self.ct = csp.tile([128, L], F16, tag="ct", name="ct")
                    self.st = csp.tile([128, L], F16, tag="st", name="st")
                    nc.sync.dma_start(
                        out=self.ct,
                        in_=cs_src["cq" if name == "q" else "ck"][:, dg, :])
                    nc.sync.dma_start(
                        out=self.st,
                        in_=cs_src["sq" if name == "q" else "sk"][:, dg, :])

                def chunk(self, lc):
                    if self.qraw is None:
                        self.qraw = rscr.tile([128, L], F16, tag="qraw",
                                              name="qraw")
                    ls = slice(lc * 512, (lc + 1) * 512)
                    ps = pq.tile([128, L], F32, tag="pq")
                    for kc in range(8):
                        nc.tensor.matmul(
                            ps[:, 0:512],
                            w_sb[self.name][:, kc,
                                            self.dg * 128:(self.dg + 1) * 128],
                            x_sb[self.name][:, kc, ls],
                            start=(kc == 0), stop=(kc == 7))
                    nc.vector.tensor_copy(self.qraw[:, ls], ps[:, 0:512])

                def finish(self):
                    dest = (qT if self.name == "q" else kT)[self.dg]
                    # z = qraw * sin (sign-folded AND pre-swapped on host);
                    # t2 = partner-swap of z via DMA copies; t1 then reuses
                    # the z slot (its readers, the swaps, are done by then)
                    z = rscr.tile([128, L], F16, tag="z")
                    nc.vector.tensor_mul(z, self.qraw, self.st)
                    t2 = rscr.tile([128, L], F16, tag="t2", name="t2")
                    for a in (0, 64):
                        nc.sync.dma_start(
                            out=t2[a:a + 32, :], in_=z[a + 32:a + 64, :])
                        nc.sync.dma_start(
                            out=t2[a + 32:a + 64, :], in_=z[a:a + 32, :])
                    t1 = rscr.tile([128, L], F16, tag="z", name="t1")
                    nc.vector.tensor_mul(t1, self.qraw, self.ct)
                    nc.vector.tensor_add(dest, t1, t2)

            def emit_projv_grp(st_i):
                # v projection for one 128-row s block; one strided cast
                # into the 66-wide v16 layout
                ps = pq.tile([128, L], F32, tag="pq", name="pv")
                for kc in range(8):
                    nc.tensor.matmul(
                        ps[:, 0:512],
                        x_sb["v"][:, kc, st_i * 128:(st_i + 1) * 128],
                        w_sb["v"][:, kc, :],
                        start=(kc == 0), stop=(kc == 7))
                src = ps.rearrange("p (t h d) -> p t h d", t=2, h=8)[:, 0]
                nc.vector.tensor_copy(v16[:, st_i, :, 0:64], src)

            class Pair:
                def __init__(self, g):
                    self.g = g              # dg == g; heads 2g, 2g+1
                    self.wx = {}            # (hh, sc) -> wx tile
                    self.wxm = {}           # (hh, mc) -> wxm tile
                    self.cs = {}            # hh -> [128, 8] colsum tile
                    self.vs = {}            # hh -> [128, 8, 66] prescaled v
                    self.pmain = {}         # hh -> [65, L] psum
                    self.pmem = {}
                    self.npm = None         # [128, L] f16: both heads numer
                    self.npx = None         # mem numer pair
                    self.bcm = None
                    self.bcx = None

            def emit_qk_pair(st, sc):
                # 4 MMs alternating row groups 0/64 (concurrent on the PE),
                # then the two exps with colsum accum
                dg = st.g
                pws = {}
                for hh in range(2):
                    pws[hh] = pq.tile([128, L], F32, tag="pq", name="pw")
                for lc in range(2):
                    for hh in range(2):
                        ho = hh * 64
                        nc.tensor.matmul(
                            pws[hh][:, lc * 512:(lc + 1) * 512],
                            kT[dg][ho:ho + 64, sc * 128:(sc + 1) * 128],
                            qT[dg][ho:ho + 64, lc * 512:(lc + 1) * 512],
                            start=True, stop=True)
                for hh in range(2):
                    if sc == 0:
                        st.cs[hh] = csacp.tile([128, 8], F32, tag="cs", name="cs")
                    wx = wexpp.tile([128, L], F16, tag="wx")
                    nc.scalar.activation(wx, pws[hh], AF.Exp,
                                         accum_out=st.cs[hh][:, sc:sc + 1])
                    st.wx[(hh, sc)] = wx

            def emit_rc_pair(st):
                # rc = 1/colsum (one batched recip per head), then all 8
                # prescaled-v stationaries in one broadcast tensor_tensor
                for hh in range(2):
                    h = st.g * 2 + hh
                    rc32 = rcp.tile([128, 8], F32, tag="rc32")
                    nc.vector.reciprocal_approx_fast(out=rc32, in_=st.cs[hh])
                    rc16 = rcp.tile([128, 8], F16, tag="rc16")
                    nc.vector.tensor_copy(rc16, rc32)
                    vs = vsp.tile([128, 8, 66], F16, tag="vs")
                    nc.vector.tensor_mul(
                        vs[:, :, 0:65], v16[:, :, h, 0:65],
                        rc16.unsqueeze(2).to_broadcast([128, 8, 65]))
                    st.vs[hh] = vs

            def emit_av_pair(st, sc):
                for hh in range(2):
                    if sc == 0:
                        st.pmain[hh] = pavp.tile([65, L], F32, tag="pav", name="pmain")
                    for lc in range(2):
                        nc.tensor.matmul(
                            st.pmain[hh][:, lc * 512:(lc + 1) * 512],
                            st.vs[hh][:, sc, 0:65],
                            st.wx[(hh, sc)][:, lc * 512:(lc + 1) * 512],
                            start=(sc == 0), stop=(sc == 7))

            def emit_memqk_pair(st, mc):
                dg = st.g
                pws = {}
                for hh in range(2):
                    pws[hh] = pq.tile([128, L], F32, tag="pq", name="pw")
                for lc in range(2):
                    for hh in range(2):
                        ho = hh * 64
                        nc.tensor.matmul(
                            pws[hh][:, lc * 512:(lc + 1) * 512],
                            kmem_sb[ho:ho + 64, dg, mc * 128:(mc + 1) * 128],
                            qT[dg][ho:ho + 64, lc * 512:(lc + 1) * 512],
                            start=True, stop=True)
                for hh in range(2):
                    wxm = wexpmp.tile([128, L], F16, tag="wxm")
                    nc.scalar.activation(wxm, pws[hh], AF.Exp)
                    st.wxm[(hh, mc)] = wxm

            def emit_memav_pair(st, mc):
                for hh in range(2):
                    h = st.g * 2 + hh
                    if mc == 0:
                        st.pmem[hh] = pavp.tile([65, L], F32, tag="pav", name="pmem")
                    for lc in range(2):
                        nc.tensor.matmul(
                            st.pmem[hh][:, lc * 512:(lc + 1) * 512],
                            vm_sb[:, h, mc, 0:65],
                            st.wxm[(hh, mc)][:, lc * 512:(lc + 1) * 512],
                            start=(mc == 0), stop=(mc == 3))

            def emit_release_main(st):
                # copies free the pair's pmain tiles for the mem phase;
                # the main-side reciprocal + broadcasts run here so only
                # the mem half trails the last memAV
                st.npm = npp.tile([128, L], F16, tag="np")
                for hh in range(2):
                    nc.vector.tensor_copy(st.npm[hh * 64:(hh + 1) * 64, :],
                                          st.pmain[hh][0:64, :])
                    nc.vector.tensor_copy(dent[hh * 32:hh * 32 + 1, :],
                                          st.pmain[hh][64:65, :])
                nc.vector.reciprocal_approx_fast(out=dent, in_=dent)
                st.bcm = bcp.tile([128, L], F16, tag="bc", name="bcm")
                for i, scr_tag in ((0, None), (1, "um")):
                    rr = rfrp.tile([1, L], F16, tag="rr", name="rr")
                    nc.vector.tensor_copy(rr, dent[i * 32:i * 32 + 1, :])
                    if scr_tag is None:
                        nc.gpsimd.partition_broadcast(st.bcm[0:64, :], rr)
                    else:
                        scr = up.tile([64, L], F16, tag=scr_tag, name="scr")
                        nc.gpsimd.partition_broadcast(scr, rr)
                        nc.vector.tensor_copy(st.bcm[64:128, :], scr)

            def emit_release_mem(st):
                st.npx = npp.tile([128, L], F16, tag="np")
                for hh in range(2):
                    nc.vector.tensor_copy(st.npx[hh * 64:(hh + 1) * 64, :],
                                          st.pmem[hh][0:64, :])
                    nc.vector.tensor_copy(dentx[hh * 32:hh * 32 + 1, :],
                                          st.pmem[hh][64:65, :])
                nc.vector.reciprocal_approx_fast(out=dentx, in_=dentx)
                st.bcx = bcp.tile([128, L], F16, tag="bc", name="bcx")
                for i, scr_tag in ((0, None), (1, "ux")):
                    rr = rfrp.tile([1, L], F16, tag="rr", name="rr")
                    nc.vector.tensor_copy(rr, dentx[i * 32:i * 32 + 1, :])
                    if scr_tag is None:
                        nc.gpsimd.partition_broadcast(st.bcx[0:64, :], rr)
                    else:
                        scr = up.tile([64, L], F16, tag=scr_tag, name="scr")
                        nc.gpsimd.partition_broadcast(scr, rr)
                        nc.vector.tensor_copy(st.bcx[64:128, :], scr)

            def emit_combine(st):
                # attn[dg] = npm * bcm + npx * bcx
                if dbg and st.g == 0:
                    nc.sync.dma_start(out=dbg_t["dbg_npm"], in_=st.npm)
                    nc.sync.dma_start(out=dbg_t["dbg_npx"], in_=st.npx)
                    nc.sync.dma_start(out=dbg_t["dbg_bcm"], in_=st.bcm)
                    nc.sync.dma_start(out=dbg_t["dbg_bcx"], in_=st.bcx)
                um = up.tile([128, L], F16, tag="um")
                nc.vector.tensor_mul(um, st.npm, st.bcm)
                ux = up.tile([128, L], F16, tag="ux")
                nc.vector.tensor_mul(ux, st.npx, st.bcx)
                nc.vector.tensor_add(attn[st.g], um, ux)
                if dbg and st.g == 0:
                    nc.sync.dma_start(out=dbg_t["dbg_attn"], in_=attn[0])

            def emit_oproj(oc, dgs, po):
                for lc in range(2):
                    for dg in dgs:
                        nc.tensor.matmul(
                            po[:, lc * 512:(lc + 1) * 512],
                            wo_sb[:, dg, oc * 128:(oc + 1) * 128],
                            attn[dg][:, lc * 512:(lc + 1) * 512],
                            start=(dg == dgs[0]), stop=(dg == dgs[-1]))

            def emit_ostage(po, oc, dest, on_act=False):
                so = ostage.tile([128, L], F16, tag="so")
                if on_act:
                    nc.scalar.copy(so, po)
                else:
                    nc.vector.tensor_copy(so, po)
                nc.sync.dma_start(out=dest[:, oc, :], in_=so)

            # ---- emission ----
            # dg0 q+k interleaved (parallel DMA rings) so QK pair 0 / the
            # exp stream starts ASAP; projv + later projections become PE
            # filler inside the periods.
            g0q = ProjG(0, "q")
            g0k = ProjG(0, "k")
            g0q.chunk(0)
            g0k.chunk(0)
            g0q.chunk(1)
            g0k.chunk(1)
            g0q.finish()
            g0k.finish()
            emit_vpath_dmas()

            fills = [(1, "q"), (1, "k"), (2, "q"),
                     (2, "k"), (3, "q"), (3, "k")]

            pairs = [Pair(g) for g in range(4)]

            # period -1: QK/memQK pair 0 + projv + fills g1
            cur = pairs[0]
            fa = fb = None
            for sc in range(8):
                if sc == 0:
                    fa = ProjG(*fills.pop(0))
                if sc == 3:
                    fb = ProjG(*fills.pop(0))
                emit_qk_pair(cur, sc)
                emit_projv_grp(sc)
                if sc == 1:
                    fa.chunk(0)
                if sc == 3:
                    fa.chunk(1)
                if sc == 4:
                    fa.finish()
                    fb.chunk(0)
                if sc == 6:
                    fb.chunk(1)
                if sc == 7:
                    fb.finish()
            emit_rc_pair(cur)
            if dbg:
                nc.sync.dma_start(out=dbg_t["dbg_qT"], in_=qT[0])
                nc.sync.dma_start(out=dbg_t["dbg_kT"], in_=kT[0])
                nc.sync.dma_start(out=dbg_t["dbg_v16"], in_=v16)
                nc.sync.dma_start(out=dbg_t["dbg_wx"], in_=cur.wx[(0, 0)])
                for hh in range(2):
                    nc.sync.dma_start(out=dbg_t["dbg_cs"][:, hh, :],
                                      in_=cur.cs[hh])
                    nc.sync.dma_start(out=dbg_t["dbg_vs"][:, hh, :, :],
                                      in_=cur.vs[hh])
            for mc in range(4):
                emit_memqk_pair(cur, mc)

            # periods 0..3
            for g in range(4):
                st = pairs[g]
                nxt = pairs[g + 1] if g + 1 < 4 else None
                fa = fb = None
                for sc in range(8):
                    if fills:
                        if sc == 0:
                            fa = ProjG(*fills.pop(0))
                        if sc == 3:
                            fb = ProjG(*fills.pop(0))
                    if nxt is not None:
                        emit_qk_pair(nxt, sc)
                    emit_av_pair(st, sc)
                    if fa is not None:
                        if sc == 1:
                            fa.chunk(0)
                        if sc == 3:
                            fa.chunk(1)
                        if sc == 4:
                            fa.finish()
                            fb.chunk(0)
                        if sc == 6:
                            fb.chunk(1)
                        if sc == 7:
                            fb.finish()
                    if g == 2 and sc % 2 == 1:
                        # out-proj pass A (dg 0+1) as PE filler
                        oc = sc // 2
                        po = pq.tile([128, L], F32, tag="pq")
                        emit_oproj(oc, (0, 1), po)
                        emit_ostage(po, oc, outT)
                    if g == 3 and sc % 2 == 1:
                        # dg2-only pass streams during period 3; staging on
                        # the (now idle) scalar engine keeps DVE free for
                        # the combine chain
                        oc = sc // 2
                        po = pq.tile([128, L], F32, tag="pq")
                        emit_oproj(oc, (2,), po)
                        emit_ostage(po, oc, outT3, on_act=True)
                if nxt is not None:
                    emit_rc_pair(nxt)
                emit_release_main(st)
                for mc in range(4):
                    emit_memav_pair(st, mc)
                    if nxt is not None:
                        emit_memqk_pair(nxt, mc)
                    if g == 2:
                        oc = 4 + mc
                        po = pq.tile([128, L], F32, tag="pq")
                        emit_oproj(oc, (0, 1), po)
                        emit_ostage(po, oc, outT)
                    if g == 3:
                        oc = 4 + mc
                        po = pq.tile([128, L], F32, tag="pq")
                        emit_oproj(oc, (2,), po)
                        emit_ostage(po, oc, outT3, on_act=True)
                emit_release_mem(st)
                emit_combine(st)
                if g == 3:
                    # tail: dg3-only matmuls after the last combine
                    for oc in range(8):
                        po = pq.tile([128, L], F32, tag="pq")
                        emit_oproj(oc, (3,), po)
                        emit_ostage(po, oc, outT2, on_act=True)
    nc.compile()
    return nc


def _perm64():
    p = np.empty(64, np.int64)
    p[:32] = np.arange(0, 64, 2)
    p[32:] = np.arange(1, 64, 2)
    return p


def _chunk(a, nchunk):
    # [C*128, F] -> [128, C, F]
    c128, f = a.shape
    return np.ascontiguousarray(
        a.reshape(nchunk, 128, f).transpose(1, 0, 2)).astype(NPF16)


def _prep_inputs(inputs):
    """Host-side shard prep. Returns list of per-core input dicts."""
    f = np.float32
    query = np.asarray(inputs["query"], f)
    key = np.asarray(inputs["key"], f)
    value = np.asarray(inputs["value"], f)
    W = np.asarray(inputs["in_proj_weight"], f)
    wo = np.asarray(inputs["out_proj_weight"], f)
    qp = np.asarray(inputs["qp"], f)
    kvp = np.asarray(inputs["kvp"], f)
    k_mem = np.asarray(inputs["k_mem"], f)
    v_mem = np.asarray(inputs["v_mem"], f)
    gate = np.asarray(inputs["gate_attn"], f)
    mask = np.asarray(inputs["mem_mask"]).astype(f)

    g = 1.0 / (1.0 + np.exp(-gate))
    p64 = _perm64()
    sgn = np.tile(np.concatenate(
        [np.full(32, -1.0, f), np.full(32, 1.0, f)]), HPC)

    # per-batch x, shared by the two cores of each batch
    xs = {}
    for n in range(N):
        xs[n] = tuple(
            _chunk(np.ascontiguousarray(t[:, n, :].T), 8)
            for t in (query, key, value))

    def swap32(x):
        y = np.empty_like(x)
        for hb in range(HPC):
            b = hb * 64
            y[b:b + 32] = x[b + 32:b + 64]
            y[b + 32:b + 64] = x[b:b + 32]
        return y

    in_maps = []
    for c in range(NC):
        n, grp = c // 2, c % 2
        dims = np.arange(grp * DCC, (grp + 1) * DCC)
        dims_perm = np.concatenate([dims[h * 64 + p64] for h in range(HPC)])
        gv = np.concatenate(
            [np.full(64, 1.0 - g[grp * HPC + h], f) for h in range(HPC)])

        wq_c = _chunk(np.ascontiguousarray(
            (W[:E][dims_perm] * np.float32(D ** -0.5)).T), 8)
        wk_c = _chunk(np.ascontiguousarray(W[E:2 * E][dims_perm].T), 8)
        wv_c = _chunk(np.ascontiguousarray(
            (W[2 * E:][dims] * gv[:, None]).T), 8)
        wo_c = _chunk(np.ascontiguousarray(wo[:, dims].T), 4)

        cq = _chunk(np.ascontiguousarray(qp[n][:, dims_perm, 0].T), 4)
        sq = _chunk(swap32(qp[n][:, dims_perm, 1].T * sgn[:, None]), 4)
        ck = _chunk(np.ascontiguousarray(kvp[n][:, dims_perm, 0].T), 4)
        sk = _chunk(swap32(kvp[n][:, dims_perm, 1].T * sgn[:, None]), 4)

        km = _chunk(np.ascontiguousarray(k_mem[n][dims_perm, :]), 4)

        vma = np.empty((HPC, 4, 128, 65), f)
        for h in range(HPC):
            vmh = (v_mem[n][dims[h * 64:(h + 1) * 64], :].T
                   * g[grp * HPC + h] * mask[n][:, None])      # [M, 64]
            vma[h, :, :, :64] = vmh.reshape(4, 128, 64)
            vma[h, :, :, 64] = mask[n].reshape(4, 128)
        vm_dev = np.ascontiguousarray(
            vma.transpose(2, 0, 1, 3)).astype(NPF16)           # [128,H,4,65]

        xq_c, xk_c, xv_c = xs[n]
        in_maps.append({
            "xq": xq_c, "xk": xk_c, "xv": xv_c,
            "wq": wq_c, "wk": wk_c, "wv": wv_c, "wo": wo_c,
            "cosq": cq, "sinq": sq, "cosk": ck, "sink": sk,
            "kmem": km, "vm": vm_dev,
        })
    return in_maps


def kernel(dbg=False, **inputs):
    key = ("nc", dbg)
    if key not in _COMPILED:
        _COMPILED[key] = _build(dbg)
    _COMPILED["nc"] = _COMPILED[key]
    nc = _COMPILED["nc"]
    in_maps = _prep_inputs(inputs)
    res = bass_utils.run_bass_kernel_spmd(nc, in_maps, core_ids=list(range(NC)))
    out = np.zeros((L, N, E), np.float64)
    for c, r in enumerate(res.results):
        n = c // 2
        oc = (r["outT"].astype(np.float64)
              + r["outT2"].astype(np.float64)
              + r["outT3"].astype(np.float64))     # [128, 8, L]
        out[:, n, :] += oc.transpose(2, 1, 0).reshape(L, E)
    out = out.astype(np.float32) + np.asarray(inputs["out_proj_bias"],
                                              np.float32)
    return out
